# revision 44
# baseline (speedup 1.0000x reference)
"""Distributed 2-layer GAT kernel for 8 Trainium2 NeuronCores — v11.

Host-side: degree-sorted relabeling, dst-major edge slots, SPMD per-core
programs keyed by the compile-time degree schedule ghat, WITHOUT self-loops
in the gather slots (the self contribution is computed from locally-kept
features).

Device-side (evolved from the v2 baseline via perfetto-trace iterations,
1299us -> ~1000us):
  * exp(leaky_relu(als+ald)) == max(Es*Ed, Es2*Ed2) with Es=exp(als),
    Es2=exp(0.2*als) precomputed per NODE in phase 0 and carried in the
    gathered table row (bf16 inside the fp8 row); Ed/Ed2 kept per own dst.
    Kills the per-edge broadcast-exp on the scalar engine (231us) and the
    per-chunk leaky/exp ops.
  * Table feats are fp8e4m3 (row: 512B feats + 16B bf16 factors, 768B
    stride; L2 row 256B) — 40% less gather DMA and AllGather traffic;
    alpha-multiply upcasts to a bf16 tile feeding the accumulation matmuls.
    Q7 descriptor rate recovers to ~8.7ns/row (ring backpressure gone).
  * Self-loops removed from the gather (2*2560 fewer descriptor rows/core);
    self term = asel*own_feat added before the softmax normalize; pad-row
    NEG masks applied via activation bias so den>0 always (no 1e-30 guard).
  * AllGather outputs in Shared DRAM (one table, not 8 replicas) — kills
    most of the AG1 dead zone.
  * Each group's AllGather trigger is deferred one gather-job into the next
    group so its semaphore wait cannot head-of-line-block the Q7 queue.
  * L2 epilogues are decoupled from AG chunking: pair-sized groups ordered
    heaviest-early (single-chunk block first, lightest last) to minimize
    the un-overlapped pipeline ramp and tail.
  * LN0 gamma/beta folded into W1 (host); LN epilogues fused
    (Square(bias=-mu, accum), scalar_tensor_tensor combos); log_softmax
    without max-shift (|z| < 1 by construction: Wo is scaled).

Known dead ends (do not revisit): single_packet=True and
prepare_only+trigger_dma both crash NRT; scalar Rsqrt/Reciprocal are
blocked by bass; full-row staging of the AG inputs regressed.
"""
import sys

sys.path.insert(0, "/opt/trn_rl_repo")

import numpy as np
import ml_dtypes

from concourse import bass, bacc, tile, mybir
from concourse import bass_utils
from concourse.masks import make_identity

BF16 = ml_dtypes.bfloat16
F32 = mybir.dt.float32
BF = mybir.dt.bfloat16
F8 = mybir.dt.float8e4
I16 = mybir.dt.int16
AF = mybir.ActivationFunctionType
OP = mybir.AluOpType

# problem constants
N, E = 20000, 320000
D_IN, HID, D_OUT = 128, 128, 32
H1, H2 = 4, 1
EPS = 1e-5

NCORES = 8
P = 128
NPAD = 20480
NBLK_G = NPAD // P
NPB = NPAD // NCORES    # 2560
NBLK = NPB // NCORES // 16 * NCORES * 16 // P if False else NPB // P  # 20
NEG = -1e9

T1COLS = 768            # L1 row (fp8): 512 feats | 8 bf16 Es,Es2 (16B) | pad
T2COLS = 256            # L2 row (fp8): 128 feats | 2 bf16 Es,Es2 (4B) | pad
KC = 16                 # max in-edge slots per gather call
NAG = 4                 # AllGather chunks
GRP = NBLK // NAG       # blocks per epilogue/AG group (5)
NSWQ = 4                # SWDGE queues
PA = 5                  # gather ring depth (gp bufs = PA + 1)

USE_SHARED_AG = True
USE_PREP = False        # prepare_only+trigger crashes NRT; inline gathers only

# ccb column-constant table layout (bf16, replicated across partitions)
CC_R1F = 0              # b_in @ W1' (512)
CC_B1C = 512            # bias1 - mean(bias1) (512)
CC_G1 = 1024            # g1 (512)
CC_B1 = 1536            # b1 (512)
CC_B2C = 2048           # bias2 - mean(bias2) (128)
CC_G2 = 2176            # g2 (128)
CC_B2 = 2304            # b2 (128)
CC_BO = 2432            # bo (32)
CC_R1A = 2464           # b_in @ W1' als|ald projections (8)
NCC = 2472


def _tid(n):
    blk = n // P
    c = blk % NCORES
    l = blk // NCORES
    cb = NBLK // NAG
    return ((l // cb) * (NPAD // NAG) + c * (NPB // NAG)
            + (l % cb) * P + n % P)


def prepare_inputs(x, edge_index):
    x = np.asarray(x, dtype=np.float32)
    ei = np.asarray(edge_index)
    src = ei[0].astype(np.int64)
    dst = ei[1].astype(np.int64)

    deg = np.bincount(dst, minlength=N)          # no self-loops
    order = np.argsort(deg, kind="stable")
    newid = np.empty(N, dtype=np.int64)
    newid[order] = np.arange(N) + (NPAD - N)

    degp = np.zeros(NPAD, dtype=np.int64)
    degp[newid] = deg
    gmax = degp.reshape(NBLK_G, P).max(axis=1)
    ghat = gmax.reshape(NBLK, NCORES).max(axis=1)
    assert (ghat > 0).all()
    S = int(P * ghat.sum())

    nd = newid[dst]
    csr_order = np.argsort(nd, kind="stable")
    nsrc_sorted = newid[src[csr_order]]
    indptr = np.zeros(NPAD + 1, dtype=np.int64)
    np.cumsum(np.bincount(nd, minlength=NPAD), out=indptr[1:])

    tid_of = _tid(np.arange(NPAD))

    goff = np.zeros(NBLK, dtype=np.int64)
    goff[1:] = np.cumsum(ghat)[:-1]

    idxw = np.zeros((NCORES, P, S // 16), dtype=np.int16)
    x_own = np.zeros((NCORES, NPB, D_IN), dtype=np.float32)
    alsfix = np.zeros((NCORES, NPB, 4), dtype=np.float32)

    inv_new = np.full(NPAD, -1, dtype=np.int64)
    inv_new[newid] = np.arange(N)

    for c in range(NCORES):
        gblk = np.arange(NBLK) * NCORES + c
        nid = (gblk[:, None] * P + np.arange(P)).reshape(-1)
        ov = inv_new[nid]
        real = ov >= 0
        x_own[c][real] = x[ov[real]]
        alsfix[c][~real, 0] = NEG        # L1 Es mask (pad rows -> Es=0)
        alsfix[c][~real, 1] = 0.2 * NEG  # L1 Es2 mask
        alsfix[c][~real, 2] = NEG        # L2 Es mask
        alsfix[c][~real, 3] = 0.2 * NEG  # L2 Es2 mask

        idx_flat = np.zeros(S, dtype=np.int16)
        for l in range(NBLK):
            d0 = nid[l * P:(l + 1) * P]
            base = goff[l] * P
            for p in range(P):
                d = d0[p]
                s0, s1 = indptr[d], indptr[d + 1]
                ks = np.arange(s1 - s0)
                idx_flat[base + ks * P + p] = tid_of[nsrc_sorted[s0:s1]]
        idxw[c] = np.tile(idx_flat.reshape(S // 16, 16).T, (NCORES, 1))

    return {
        "ghat": [int(g) for g in ghat],
        "S": S,
        "idxw": idxw,
        "x_own": x_own,
        "alsfix": alsfix,
        "newid": newid,
    }


def prepare_weights(W1, att1_s, att1_d, bias1, g1, b1, g_in, b_in,
                    W2, att2_s, att2_d, bias2, g2, b2, Wo, bo):
    W1 = np.asarray(W1, np.float32)
    W2 = np.asarray(W2, np.float32)
    g_in = np.asarray(g_in, np.float32)
    b_in = np.asarray(b_in, np.float32)

    w1e = np.zeros((D_IN, 520), dtype=np.float32)
    w1e[:, :512] = W1
    W1h = W1.reshape(D_IN, H1, HID)
    w1e[:, 512:516] = np.einsum("khc,hc->kh", W1h, np.asarray(att1_s, np.float32))
    w1e[:, 516:520] = np.einsum("khc,hc->kh", W1h, np.asarray(att1_d, np.float32))
    # fold LN0's gamma into W1 (rows) and beta into a constant output row
    w1f = w1e * g_in[:, None]
    r1 = b_in @ w1e                      # (520,)
    w1ext = w1f.astype(BF16)

    w2e = np.zeros((4 * HID, 130), dtype=np.float32)
    w2e[:, :128] = W2
    w2e[:, 128] = W2 @ np.asarray(att2_s, np.float32)[0]
    w2e[:, 129] = W2 @ np.asarray(att2_d, np.float32)[0]
    w2ext = np.ascontiguousarray(
        w2e.reshape(4, P, 130).transpose(1, 0, 2)).astype(BF16)

    woext = np.asarray(Wo, np.float32).astype(BF16)

    bias1 = np.asarray(bias1, np.float32)
    bias2 = np.asarray(bias2, np.float32)
    cc = np.zeros(NCC, dtype=np.float32)
    cc[CC_R1F:CC_R1F + 512] = r1[:512]
    cc[CC_B1C:CC_B1C + 512] = bias1 - bias1.mean()
    cc[CC_G1:CC_G1 + 512] = g1
    cc[CC_B1:CC_B1 + 512] = b1
    cc[CC_B2C:CC_B2C + 128] = bias2 - bias2.mean()
    cc[CC_G2:CC_G2 + 128] = g2
    cc[CC_B2:CC_B2 + 128] = b2
    cc[CC_BO:CC_BO + 32] = bo
    cc[CC_R1A:CC_R1A + 8] = r1[512:520]
    colconst = np.tile(cc[None, :], (P, 1))

    return {"w1ext": w1ext, "w2ext": w2ext.reshape(P, 4 * 130),
            "woext": woext, "colconst": colconst}


def _bap(ap, dims):
    return bass.AP(ap.tensor, ap.offset, [ap.ap[0]] + [list(d) for d in dims])


def build_program(ghat, num_devices=NCORES):
    S = int(P * sum(ghat))
    goff = np.zeros(NBLK, dtype=np.int64)
    goff[1:] = np.cumsum(ghat)[:-1]

    nc = bacc.Bacc("TRN2", target_bir_lowering=False, debug=False,
                   num_devices=num_devices, num_swdge_queues=NSWQ)

    x_own = nc.dram_tensor("x_own", [NPB, D_IN], F32, kind="ExternalInput")
    idxw = nc.dram_tensor("idxw", [P, S // 16], I16, kind="ExternalInput")
    alsfix = nc.dram_tensor("alsfix", [NPB, 4], F32, kind="ExternalInput")
    w1ext = nc.dram_tensor("w1ext", [D_IN, 520], BF, kind="ExternalInput")
    w2ext = nc.dram_tensor("w2ext", [P, 4 * 130], BF, kind="ExternalInput")
    woext = nc.dram_tensor("woext", [P, D_OUT], BF, kind="ExternalInput")
    colconst = nc.dram_tensor("colconst", [P, NCC], BF, kind="ExternalInput")
    out = nc.dram_tensor("out", [NPB, D_OUT], F32, kind="ExternalOutput")

    if USE_SHARED_AG:
        ag1_out_t = nc.dram_tensor("ag1o", [NPAD, T1COLS], F8,
                                   kind="Internal", addr_space="Shared")
        ag2_out_t = nc.dram_tensor("ag2o", [NPAD, T2COLS], F8,
                                   kind="Internal", addr_space="Shared")

    rg = [list(range(num_devices))]

    # processing order for layer 1: heaviest group first
    grp_w = [sum(ghat[g * GRP:(g + 1) * GRP]) for g in range(NAG)]
    grp_order2 = sorted(range(NAG), key=lambda g: -grp_w[g])

    with tile.TileContext(nc) as tc:
        with (
            tc.tile_pool(name="cst", bufs=1) as cst,
            tc.tile_pool(name="wp", bufs=2) as wp,
            tc.tile_pool(name="wq", bufs=2) as wq,
            tc.tile_pool(name="hp", bufs=1) as hp,
            tc.tile_pool(name="gp", bufs=PA + 1) as gp,
            tc.tile_pool(name="ps", bufs=2, space="PSUM") as ps,
            tc.tile_pool(name="pss", bufs=2, space="PSUM") as pss,
            tc.tile_pool(name="dram", bufs=1, space="DRAM") as dram,
        ):
            # ---- constants (idx first: unblocks gather preps) ----
            idx_sb = cst.tile([P, S // 16], I16)
            nc.sync.dma_start(idx_sb[:], idxw[:])
            ident = cst.tile([P, P], BF)
            make_identity(nc, ident[:])
            w1s = cst.tile([P, 520], BF)
            nc.sync.dma_start(w1s[:], w1ext[:])
            w2s = cst.tile([P, 4, 130], BF)
            nc.sync.dma_start(w2s[:], w2ext[:])
            wos = cst.tile([P, D_OUT], BF)
            nc.sync.dma_start(wos[:], woext[:])
            ccb = cst.tile([P, NCC], BF)
            nc.sync.dma_start(ccb[:], colconst[:])
            afix = cst.tile([P, NBLK, 4], F32)
            nc.sync.dma_start(
                afix[:], bass.AP(alsfix.ap().tensor, 0,
                                 [[4, P], [4 * P, NBLK], [1, 4]]))
            eps_t = cst.tile([P, 1], F32)
            nc.vector.memset(eps_t[:], EPS)

            # per-own-dst runtime tables
            ownf = cst.tile([P, NBLK, 512], F8)     # own L1 feats
            edt1 = cst.tile([P, NBLK, H1], F32)     # Ed   (L1)
            edt12 = cst.tile([P, NBLK, H1], F32)    # Ed2  (L1)
            asel1 = cst.tile([P, NBLK, H1], F32)    # self alpha (L1)
            trow2 = cst.tile([P, NBLK, 132], F8)    # own L2 feats+Es+Es2
            edt2 = cst.tile([P, NBLK, 1], F32)
            edt22 = cst.tile([P, NBLK, 1], F32)
            asel2 = cst.tile([P, NBLK, 1], F32)

            ag1_in = dram.tile([NPB, T1COLS], F8)
            ag2_in = dram.tile([NPB, T2COLS], F8)
            if USE_SHARED_AG:
                ag1_out = ag1_out_t
                ag2_out = ag2_out_t
            else:
                ag1_out = dram.tile([NPAD, T1COLS], F8)
                ag2_out = dram.tile([NPAD, T2COLS], F8)

            dsem = [nc.alloc_semaphore(f"gdma{q}") for q in range(NSWQ)]
            ntrig = [0] * NSWQ          # triggered gathers per queue

            def transpose_to(dst_bf, src_bf):
                pst = pss.tile([P, P], BF, tag="tp")
                nc.tensor.transpose(out=pst[:], in_=src_bf, identity=ident[:])
                nc.vector.tensor_copy(out=dst_bf, in_=pst[:])

            # ---------- phase 0: LN0 + W1 matmul, build L1 table shard ----
            def phase0():
                for t in range(NBLK):
                    xt_t = wp.tile([P, D_IN], F32, tag="xt")
                    nc.sync.dma_start(xt_t[:], x_own[t * P:(t + 1) * P, :])
                    xt = xt_t[:]
                    mu = wp.tile([P, 1], F32, tag="p0mu")
                    nc.vector.tensor_reduce(out=mu[:], in_=xt,
                                            axis=mybir.AxisListType.X, op=OP.add)
                    nc.scalar.mul(mu[:], mu[:], -1.0 / D_IN)
                    sq = wp.tile([P, D_IN], BF, tag="p0sq")
                    ss = wp.tile([P, 1], F32, tag="p0ss")
                    nc.scalar.activation(sq[:], xt, AF.Square,
                                         bias=mu[:], accum_out=ss[:])
                    rstd = wp.tile([P, 1], F32, tag="p0rs")
                    nc.scalar.activation(rstd[:], ss[:], AF.Sqrt,
                                         bias=eps_t[:], scale=1.0 / D_IN)
                    nc.vector.reciprocal(rstd[:], rstd[:])
                    xnb = wp.tile([P, D_IN], BF, tag="xnb")
                    nc.vector.scalar_tensor_tensor(
                        out=xnb[:], in0=xt, scalar=mu[:],
                        in1=_bap(rstd[:], [(0, D_IN)]),
                        op0=OP.add, op1=OP.mult)
                    xT = wp.tile([P, P], BF, tag="xT")
                    transpose_to(xT[:], xnb[:])
                    ps1 = ps.tile([P, 512], F32, tag="big")
                    nc.tensor.matmul(ps1[:], lhsT=xT[:], rhs=w1s[:, 0:512],
                                     start=True, stop=True)
                    ps2_t = pss.tile([P, 130], F32, tag="mm2")
                    ps2 = ps2_t[:, 0:8]
                    nc.tensor.matmul(ps2[:], lhsT=xT[:], rhs=w1s[:, 512:520],
                                     start=True, stop=True)
                    # feats (+r1) into own table
                    nc.vector.tensor_tensor(
                        out=ownf[:, t, :], in0=ps1[:],
                        in1=ccb[:, CC_R1F:CC_R1F + 512], op=OP.add)
                    # als | ald (+ b_in@W1 fold); pad NEG applied via bias
                    alf = wp.tile([P, 8], F32, tag="alf")
                    nc.vector.tensor_tensor(out=alf[:], in0=ps2[:, 0:8],
                                            in1=ccb[:, CC_R1A:CC_R1A + 8],
                                            op=OP.add)
                    fct = wp.tile([P, 8], BF, tag="fct")
                    nc.scalar.activation(fct[:, 0:4], alf[:, 0:4], AF.Exp,
                                         bias=afix[:, t, 0:1])
                    nc.scalar.activation(fct[:, 4:8], alf[:, 0:4], AF.Exp,
                                         scale=0.2, bias=afix[:, t, 1:2])
                    nc.scalar.activation(edt1[:, t, :], alf[:, 4:8], AF.Exp)
                    nc.scalar.activation(edt12[:, t, :], alf[:, 4:8], AF.Exp,
                                         scale=0.2)
                    us = wp.tile([P, 4], F32, tag="us")
                    nc.vector.tensor_tensor(out=us[:], in0=alf[:, 0:4],
                                            in1=alf[:, 4:8], op=OP.add)
                    nc.vector.scalar_tensor_tensor(
                        out=us[:], in0=us[:], scalar=0.2, in1=us[:],
                        op0=OP.mult, op1=OP.max)
                    nc.scalar.activation(asel1[:, t, :], us[:], AF.Exp)
                    nc.sync.dma_start(ag1_in[t * P:(t + 1) * P, 0:512],
                                      ownf[:, t, :])
                    nc.sync.dma_start(
                        ag1_in[t * P:(t + 1) * P, 512:528].bitcast(BF),
                        fct[:])
                    if t % GRP == GRP - 1:
                        j = t // GRP
                        r0, r1_ = j * NPB // NAG, (j + 1) * NPB // NAG
                        nc.gpsimd.collective_compute(
                            "AllGather", OP.bypass, replica_groups=rg,
                            ins=[ag1_in[r0:r1_, :].opt()],
                            outs=[ag1_out[j * NPAD // NAG:
                                          (j + 1) * NPAD // NAG, :].opt()])

            # ---------- gather pipeline machinery ----------
            jobs = []   # list of (prep_fn, compute_fn)
            qrr = [0]

            def run_jobs():
                n = len(jobs)
                if not USE_PREP:
                    for i in range(n):
                        jobs[i][0]()
                        jobs[i][1]()
                else:
                    for i in range(n + PA):
                        if i < n:
                            jobs[i][0]()
                        j = i - PA
                        if 0 <= j < n:
                            jobs[j][1]()
                jobs.clear()

            # ---------- GAT layer (shared for L1/L2) ----------
            def gat_layer(layer, blocks_by_group, tcols, heads, epilogue,
                          epi_defer=1):
                """Queue gather+aggregate jobs.  Each group's epilogue (which
                ends in an AllGather trigger on the gpsimd queue) is deferred
                `epi_defer` jobs into the NEXT group so its semaphore wait
                doesn't head-of-line-block the next group's gathers on Q7."""
                tag = f"g{layer}"
                table = (ag1_out[0:NPAD, :] if layer == 1
                         else ag2_out[0:NPAD, :])
                state = {}

                pending = []    # (grp, at_job_index) for deferred epilogues
                for gi, grp in enumerate(blocks_by_group):
                    for bi, l in enumerate(grp):
                        g = ghat[l]
                        chunks = []
                        k0 = 0
                        while k0 < g:
                            kn = min(KC, g - k0)
                            chunks.append((k0, kn))
                            k0 += kn
                        for ci, (k0, kn) in enumerate(chunks):
                            jobs.append(_make_job(
                                layer, tag, table, l, k0, kn,
                                ci == 0, k0 + kn >= g, tcols, heads, state,
                                None, grp))
                    pending.append((grp, len(jobs) + epi_defer - 1))
                # attach deferred epilogues (clamped to the final job)
                for grp, at in pending:
                    at = min(at, len(jobs) - 1)
                    prep_fn, comp_fn = jobs[at]
                    def with_epi(comp_fn=comp_fn, grp=grp):
                        comp_fn()
                        epilogue(grp, state)
                    jobs[at] = (prep_fn, with_epi)

            def _make_job(layer, tag, table, l, k0, kn, first, last,
                          tcols, heads, state, epi, grp):
                # single shared gather ring: L2's 256-col rows live in the
                # same [P, KC, T1COLS] tiles (contiguous rows, smaller view)
                gt_t = gp.tile([P, KC, T1COLS], F8, tag="g")
                gbase = gt_t[:]

                def gap(off, dims):
                    return bass.AP(gbase.tensor, gbase.offset + off,
                                   [gbase.ap[0]] + [list(d) for d in dims])

                q = qrr[0] % NSWQ
                qrr[0] += 1
                edA = edt1 if layer == 1 else edt2
                edB = edt12 if layer == 1 else edt22
                C = 512 if layer == 1 else 128
                eo = C                      # factor byte offset in the row
                nf8 = 2 * heads             # bytes per factor set (bf16)

                def prep():
                    if USE_PREP:
                        nc.gpsimd.dma_gather(
                            gap(0, [(tcols, kn), (1, tcols)]), table,
                            idx_sb[:, 8 * (int(goff[l]) + k0):
                                   8 * (int(goff[l]) + k0 + kn)],
                            P * kn, P * kn, tcols, prepare_only=True,
                            sem=dsem[q], single_packet=False, queue_num=q)
                    else:
                        nc.gpsimd.dma_gather(
                            gap(0, [(tcols, kn), (1, tcols)]), table,
                            idx_sb[:, 8 * (int(goff[l]) + k0):
                                   8 * (int(goff[l]) + k0 + kn)],
                            P * kn, P * kn, tcols,
                            single_packet=False, queue_num=q)

                def compute():
                    if USE_PREP:
                        nc.gpsimd.trigger_dma(count=None, queue_num=q)
                        ntrig[q] += 1
                        nc.vector.wait_ge(dsem[q], 16 * ntrig[q])
                    # alpha = max(Es*Ed, Es2*Ed2); factors are bf16 inside
                    # the fp8 row -> bitcast views
                    t1 = wp.tile([P, KC, heads], F32, tag=f"t1{layer}")
                    nc.vector.tensor_tensor(
                        out=t1[:, 0:kn, :],
                        in0=gap(eo, [(tcols, kn), (1, nf8)]).bitcast(BF),
                        in1=_bap(edA[:, l, :], [(0, kn), (1, heads)]),
                        op=OP.mult)
                    t2 = wp.tile([P, KC, heads], F32, tag=f"t2{layer}")
                    nc.vector.tensor_tensor(
                        out=t2[:, 0:kn, :],
                        in0=gap(eo + nf8, [(tcols, kn),
                                           (1, nf8)]).bitcast(BF),
                        in1=_bap(edB[:, l, :], [(0, kn), (1, heads)]),
                        op=OP.mult)
                    aw = wp.tile([P, KC, heads], BF, tag=f"aw{layer}")
                    nc.vector.tensor_tensor(out=aw[:, 0:kn, :],
                                            in0=t1[:, 0:kn, :],
                                            in1=t2[:, 0:kn, :], op=OP.max)
                    if first:
                        psA_t = ps.tile([P, 512], F32, tag="big",
                                        name=f"psA{layer}_{l}")
                        den_t = wp.tile([P, heads], F32, tag=f"den{layer}",
                                        name=f"den{layer}_{l}")
                        state["psA"] = psA_t
                        state["den"] = den_t
                    psA = state["psA"][:, 0:C]
                    den = state["den"]
                    dt_ = wp.tile([P, heads], F32, tag=f"dt{layer}")
                    red = den if k0 == 0 else dt_
                    if heads > 1:
                        nc.vector.tensor_reduce(
                            out=red[:],
                            in_=_bap(aw[:], [(1, heads), (heads, kn)]),
                            axis=mybir.AxisListType.X, op=OP.add)
                    else:
                        nc.vector.tensor_reduce(
                            out=red[:], in_=_bap(aw[:], [(1, kn)]),
                            axis=mybir.AxisListType.X, op=OP.add)
                    if k0:
                        nc.vector.tensor_add(den[:], den[:], dt_[:])
                    # alpha * feats: fp8 rows -> bf16 w tile
                    w = wq.tile([P, KC, 512], BF, tag="w")
                    wb = w[:]
                    if heads > 1:
                        nc.vector.tensor_tensor(
                            out=_bap(wb, [(512, kn), (HID, heads),
                                          (1, HID)]),
                            in0=gap(0, [(tcols, kn), (HID, heads),
                                        (1, HID)]),
                            in1=_bap(aw[:], [(heads, kn), (1, heads),
                                             (0, HID)]),
                            op=OP.mult)
                    else:
                        nc.vector.tensor_tensor(
                            out=_bap(wb, [(512, kn), (1, C)]),
                            in0=gap(0, [(tcols, kn), (1, C)]),
                            in1=_bap(aw[:], [(1, kn), (0, C)]),
                            op=OP.mult)
                    for k in range(kn):
                        nc.tensor.matmul(
                            psA[:], lhsT=ident[:],
                            rhs=bass.AP(wb.tensor, wb.offset + k * 512,
                                        [wb.ap[0], [1, C]]),
                            start=(k0 + k == 0),
                            stop=(k0 + k == ghat[l] - 1))
                    if last:
                        # den += self alpha
                        aself = (asel1 if layer == 1 else asel2)
                        nc.vector.tensor_tensor(out=den[:], in0=den[:],
                                                in1=aself[:, l, :], op=OP.add)
                        state[f"den{l}"] = state.pop("den")
                        state[f"psA{l}"] = state.pop("psA")
                    if epi is not None:
                        epi(grp, state)

                return prep, compute

            # ---------- batched epilogue: layer 1 ----------
            def epi1(grp, state):
                gi = grp[0] // GRP
                h1bs = []
                for l in grp:
                    den = state.pop(f"den{l}")
                    psA = state.pop(f"psA{l}")
                    denr = wp.tile([P, H1], F32, tag="dr1")
                    nc.vector.reciprocal(denr[:], den[:])
                    tmp = wp.tile([P, 512], BF, tag="tmp1")
                    nc.vector.tensor_tensor(
                        out=_bap(tmp[:], [(HID, H1), (1, HID)]),
                        in0=_bap(ownf[:, l, :], [(HID, H1), (1, HID)]),
                        in1=_bap(asel1[:, l, :], [(1, H1), (0, HID)]),
                        op=OP.mult)
                    acc = wp.tile([P, 512], F32, tag="acc1")
                    nc.vector.tensor_tensor(out=acc[:], in0=psA[:, 0:512],
                                            in1=tmp[:], op=OP.add)
                    hb = hp.tile([P, 512], BF, tag=f"h1_{l % GRP}")
                    nc.vector.tensor_tensor(
                        out=_bap(hb[:], [(HID, H1), (1, HID)]),
                        in0=_bap(acc[:], [(HID, H1), (1, HID)]),
                        in1=_bap(denr[:], [(1, H1), (0, HID)]),
                        op=OP.mult)
                    # fused LN
                    mu = wp.tile([P, 1], F32, tag="e1mu")
                    nc.vector.tensor_reduce(out=mu[:], in_=hb[:],
                                            axis=mybir.AxisListType.X,
                                            op=OP.add)
                    nc.scalar.mul(mu[:], mu[:], -1.0 / 512)
                    xc = wp.tile([P, 512], BF, tag=f"e1xc{l % GRP}",
                                 name=f"e1xc{l}")
                    nc.vector.scalar_tensor_tensor(
                        out=xc[:], in0=hb[:], scalar=mu[:],
                        in1=ccb[:, CC_B1C:CC_B1C + 512],
                        op0=OP.add, op1=OP.add)
                    sq = wp.tile([P, 512], BF, tag="e1sq")
                    ss = wp.tile([P, 1], F32, tag=f"e1ss{l % GRP}",
                                 name=f"e1ss{l}")
                    nc.scalar.activation(sq[:], xc[:], AF.Square,
                                         accum_out=ss[:])
                    rstd = wp.tile([P, 1], F32, tag=f"e1rs{l % GRP}",
                                   name=f"e1rs{l}")
                    nc.scalar.activation(rstd[:], ss[:], AF.Sqrt,
                                         bias=eps_t[:], scale=1.0 / 512)
                    nc.vector.reciprocal(rstd[:], rstd[:])
                    y = wp.tile([P, 512], BF, tag=f"e1y{l % GRP}",
                                name=f"e1y{l}")
                    nc.vector.scalar_tensor_tensor(
                        out=y[:], in0=xc[:], scalar=rstd[:],
                        in1=ccb[:, CC_G1:CC_G1 + 512],
                        op0=OP.mult, op1=OP.mult)
                    nc.vector.tensor_tensor(out=y[:], in0=y[:],
                                            in1=ccb[:, CC_B1:CC_B1 + 512],
                                            op=OP.add)
                    h1bs.append((l, y))
                # ACT table: Gelu (batched)
                outs = []
                for bi, (l, y) in enumerate(h1bs):
                    h1b = wp.tile([P, 512], BF, tag=f"h1b{bi}",
                                  name=f"h1b{bi}")
                    nc.scalar.activation(h1b[:], y[:], AF.Gelu)
                    outs.append((l, h1b))
                # W2 matmul + t2 table build + AG2 chunk
                for l, h1b in outs:
                    ps3 = pss.tile([P, 130], F32, tag="mm2")
                    for cch in range(4):
                        hT = wp.tile([P, P], BF, tag="hT")
                        transpose_to(hT[:], h1b[:, cch * P:(cch + 1) * P])
                        nc.tensor.matmul(ps3[:], lhsT=hT[:],
                                         rhs=w2s[:, cch, :],
                                         start=(cch == 0), stop=(cch == 3))
                    nc.vector.tensor_copy(out=trow2[:, l, 0:128],
                                          in_=ps3[:, 0:128])
                    alf2 = wp.tile([P, 2], F32, tag="alf2")
                    nc.vector.tensor_copy(out=alf2[:], in_=ps3[:, 128:130])
                    nc.scalar.activation(trow2[:, l, 128:130].bitcast(BF),
                                         alf2[:, 0:1], AF.Exp,
                                         bias=afix[:, l, 2:3])
                    nc.scalar.activation(trow2[:, l, 130:132].bitcast(BF),
                                         alf2[:, 0:1], AF.Exp,
                                         scale=0.2, bias=afix[:, l, 3:4])
                    nc.scalar.activation(edt2[:, l, :], alf2[:, 1:2],
                                         AF.Exp)
                    nc.scalar.activation(edt22[:, l, :], alf2[:, 1:2],
                                         AF.Exp, scale=0.2)
                    us2 = wp.tile([P, 1], F32, tag="us2")
                    nc.vector.tensor_tensor(out=us2[:], in0=alf2[:, 0:1],
                                            in1=alf2[:, 1:2], op=OP.add)
                    nc.vector.scalar_tensor_tensor(
                        out=us2[:], in0=us2[:], scalar=0.2, in1=us2[:],
                        op0=OP.mult, op1=OP.max)
                    nc.scalar.activation(asel2[:, l, :], us2[:], AF.Exp)
                    nc.sync.dma_start(ag2_in[l * P:(l + 1) * P, 0:132],
                                      trow2[:, l, :])
                r0, r1_ = gi * NPB // NAG, (gi + 1) * NPB // NAG
                nc.gpsimd.collective_compute(
                    "AllGather", OP.bypass, replica_groups=rg,
                    ins=[ag2_in[r0:r1_, :].opt()],
                    outs=[ag2_out[gi * NPAD // NAG:
                                  (gi + 1) * NPAD // NAG, :].opt()])

            # ---------- batched epilogue: layer 2 + output head ----------
            def epi2(grp, state):
                ys = []
                for l in grp:
                    den = state.pop(f"den{l}")
                    psA = state.pop(f"psA{l}")
                    denr = wp.tile([P, 1], F32, tag="dr2")
                    nc.vector.reciprocal(denr[:], den[:])
                    tmp = wp.tile([P, 128], BF, tag="tmp2")
                    nc.vector.tensor_tensor(
                        out=tmp[:], in0=trow2[:, l, 0:128],
                        in1=_bap(asel2[:, l, :], [(0, 128)]),
                        op=OP.mult)
                    acc = wp.tile([P, 128], F32, tag="acc2")
                    nc.vector.tensor_tensor(out=acc[:], in0=psA[:, 0:128],
                                            in1=tmp[:], op=OP.add)
                    hb = wp.tile([P, 128], BF, tag=f"h2_{l % GRP}",
                                 name=f"h2_{l}")
                    nc.vector.tensor_scalar_mul(out=hb[:], in0=acc[:],
                                                scalar1=denr[:])
                    mu = wp.tile([P, 1], F32, tag="e2mu")
                    nc.vector.tensor_reduce(out=mu[:], in_=hb[:],
                                            axis=mybir.AxisListType.X,
                                            op=OP.add)
                    nc.scalar.mul(mu[:], mu[:], -1.0 / 128)
                    xc = wp.tile([P, 128], BF, tag=f"e2xc{l % GRP}",
                                 name=f"e2xc{l}")
                    nc.vector.scalar_tensor_tensor(
                        out=xc[:], in0=hb[:], scalar=mu[:],
                        in1=ccb[:, CC_B2C:CC_B2C + 128],
                        op0=OP.add, op1=OP.add)
                    sq = wp.tile([P, 128], BF, tag="e2sq")
                    ss = wp.tile([P, 1], F32, tag=f"e2ss{l % GRP}",
                                 name=f"e2ss{l}")
                    nc.scalar.activation(sq[:], xc[:], AF.Square,
                                         accum_out=ss[:])
                    rstd = wp.tile([P, 1], F32, tag=f"e2rs{l % GRP}",
                                   name=f"e2rs{l}")
                    nc.scalar.activation(rstd[:], ss[:], AF.Sqrt,
                                         bias=eps_t[:], scale=1.0 / 128)
                    nc.vector.reciprocal(rstd[:], rstd[:])
                    y = wp.tile([P, 128], BF, tag=f"e2y{l % GRP}",
                                name=f"e2y{l}")
                    nc.vector.scalar_tensor_tensor(
                        out=y[:], in0=xc[:], scalar=rstd[:],
                        in1=ccb[:, CC_G2:CC_G2 + 128],
                        op0=OP.mult, op1=OP.mult)
                    nc.vector.tensor_tensor(out=y[:], in0=y[:],
                                            in1=ccb[:, CC_B2:CC_B2 + 128],
                                            op=OP.add)
                    ys.append((l, y))
                h2bs = []
                for bi, (l, y) in enumerate(ys):
                    h2b = wp.tile([P, 128], BF, tag=f"h2b{bi}",
                                  name=f"h2b{bi}")
                    nc.scalar.activation(h2b[:], y[:], AF.Gelu)
                    h2bs.append((l, h2b))
                zs = []
                for l, h2b in h2bs:
                    hoT = wp.tile([P, P], BF, tag="hoT")
                    transpose_to(hoT[:], h2b[:])
                    pso_t = pss.tile([P, 130], F32, tag="mm2")
                    pso = pso_t[:, 0:D_OUT]
                    nc.tensor.matmul(pso[:], lhsT=hoT[:], rhs=wos[:],
                                     start=True, stop=True)
                    z = hp.tile([P, D_OUT], F32, tag=f"z_{len(zs)}")
                    nc.vector.tensor_tensor(out=z[:], in0=pso[:],
                                            in1=ccb[:, CC_BO:CC_BO + 32],
                                            op=OP.add)
                    zs.append((l, z))
                # log-softmax (|z| < 1: no max-shift needed)
                sds = []
                for bi, (l, z) in enumerate(zs):
                    ez = wp.tile([P, D_OUT], BF, tag="ez")
                    sden = wp.tile([P, 1], F32, tag=f"sden{bi}",
                                   name=f"sden{bi}")
                    nc.scalar.activation(ez[:], z[:], AF.Exp,
                                         accum_out=sden[:])
                    sds.append(sden)
                lnds = []
                for bi, sden in enumerate(sds):
                    lnd = wp.tile([P, 1], F32, tag=f"lnd{bi}",
                                  name=f"lnd{bi}")
                    nc.scalar.activation(lnd[:], sden[:], AF.Ln)
                    lnds.append(lnd)
                for (l, z), lnd in zip(zs, lnds):
                    res = wp.tile([P, D_OUT], F32, tag="res")
                    nc.vector.tensor_scalar_sub(out=res[:], in0=z[:],
                                                scalar1=lnd[:])
                    nc.sync.dma_start(out[l * P:(l + 1) * P, :], res[:])

            # ---------- emit ----------
            # NOTE: preps capture their gather-source deps at emission time,
            # so every AllGather writing a table must be emitted before the
            # first prep that reads it (layer pipelines run separately).
            phase0()
            groups2 = [[g * GRP + i for i in range(GRP)] for g in grp_order2]
            gat_layer(1, groups2, T1COLS, H1, epi1)
            run_jobs()
            # L2 epilogues need no AllGather: pair-sized groups (amortize
            # ACT-table loads, small un-overlapped tail), heaviest blocks
            # early but a single-chunk block first (short pipeline ramp),
            # lightest last
            order2 = sorted(range(NBLK), key=lambda l: -ghat[l])
            if len(order2) > 1:
                order2[0], order2[1] = order2[1], order2[0]
            groups4 = [order2[i:i + 2] for i in range(0, NBLK, 2)]
            gat_layer(2, groups4, T2COLS, H2, epi2)
            run_jobs()

    nc.compile()
    return nc


_CACHE = {}
_LAST_RUN = {}


def kernel(x, edge_index, g_in, b_in, W1, att1_s, att1_d, bias1, g1, b1,
           W2, att2_s, att2_d, bias2, g2, b2, Wo, bo):
    prep = prepare_inputs(x, edge_index)
    wts = prepare_weights(W1, att1_s, att1_d, bias1, g1, b1, g_in, b_in,
                          W2, att2_s, att2_d, bias2, g2, b2, Wo, bo)

    key = tuple(prep["ghat"])
    if key not in _CACHE:
        _CACHE[key] = build_program(prep["ghat"])
    nc = _CACHE[key]

    in_maps = []
    for c in range(NCORES):
        in_maps.append({
            "x_own": prep["x_own"][c],
            "idxw": prep["idxw"][c],
            "alsfix": prep["alsfix"][c],
            "w1ext": wts["w1ext"],
            "w2ext": wts["w2ext"].astype(BF16),
            "woext": wts["woext"],
            "colconst": wts["colconst"].astype(BF16),
        })

    _LAST_RUN.update(nc=nc, in_maps=in_maps, prep=prep)
    res = bass_utils.run_bass_kernel_spmd(nc, in_maps,
                                          core_ids=list(range(NCORES)))
    outs = [res.results[c]["out"] for c in range(NCORES)]

    newid = prep["newid"]
    blk = newid // P
    core = blk % NCORES
    row = (blk // NCORES) * P + newid % P
    full = np.empty((N, D_OUT), dtype=np.float32)
    for c in range(NCORES):
        sel = core == c
        full[sel] = outs[c][row[sel]]
    return full
